# revision 1
# baseline (speedup 1.0000x reference)
"""BesselConv2d Trainium2 kernel.

Math (matches reference):
  wr = T_real @ w_r - T_imag @ w_i          (M, K^2, Cin*Cout)
  wi = T_real @ w_i + T_imag @ w_r
  Wf = einops to (2*M*Cout, Cin, 9, 9) filter bank
  y  = conv2d(x, Wf, SAME)                  (N, 2048, 64, 64)
  out = square(y).reshape(N,2,M,Cout,H,W).sum((1,2)) + b

Device strategy (8 cores, data-parallel over batch: 4 images/core):
  Direct conv as 45 PSUM-accumulated fp32r matmuls per (octile, pixel-tile):
  K = 128 = 64 ci x 2 kx-shifts (image planes stored twice in SBUF, second
  copy pre-shifted +1 col), M = 128 output channels (2 cm-values x 64 cout),
  N = 512 pixels (8 rows x 64 cols), rhs = strided window AP into the padded
  plane. 9 ky x 5 kx-pairs = 45 matmuls (kx=9 lane is zero-weighted pad).
  Square on ScalarE, accumulate over the 16 octiles on VectorE, fold the
  2 cm partition-halves + bias, DMA out.

Weight prep (filter bank + matmul-tile layout) is done host-side in numpy.
"""

import numpy as np

N_CORES = 8
N, CIN, H, W = 32, 64, 64, 64
COUT = 64
M_FREQ = 16
K = 9
PW = 72                 # padded plane width/height (64 + 2*4)
XW = 5248               # plane row-buffer width (>= max window slice 5192)
WO = 45 * 128           # weight elems per octile per partition row
NIMG = N // N_CORES     # images per core
NOCT = 16               # octiles of 128 output channels (2048 total)


def _host_prep(x, T_real, T_imag, w_r, w_i, b):
    # filter bank, exactly as the reference builds it
    wr = np.matmul(T_real, w_r) - np.matmul(T_imag, w_i)
    wi = np.matmul(T_real, w_i) + np.matmul(T_imag, w_r)
    Wf = np.stack([wr, wi], axis=0).reshape(2, M_FREQ, K, K, CIN, COUT)
    Wf = Wf.transpose(0, 1, 5, 4, 2, 3).reshape(2 * M_FREQ * COUT, CIN, K, K)

    # matmul-tile layout:
    # Wsb[o, s*64+ci, ky*5+g, q*64+cout] = Wv[2o+q, cout, ci, ky, 2g+s], kx=2g+s<=8
    Wv = Wf.reshape(32, COUT, CIN, K, K)          # (cm, cout, ci, ky, kx)
    Wv10 = np.zeros((32, COUT, CIN, K, 10), np.float32)
    Wv10[..., :K] = Wv
    Wp = Wv10.reshape(16, 2, COUT, CIN, K, 5, 2).transpose(0, 6, 3, 4, 5, 1, 2)
    Wsb = np.ascontiguousarray(Wp).reshape(NOCT, 128, 45, 128)
    w_flat = np.ascontiguousarray(
        Wsb.reshape(NOCT, 128, WO).transpose(1, 0, 2)).reshape(128, NOCT * WO)

    # padded planes, flattened rows
    xpad = np.zeros((N, CIN, PW, PW), np.float32)
    xpad[:, :, 4:68, 4:68] = x
    xflat = np.zeros((N, CIN, XW), np.float32)
    xflat[:, :, 0:PW * PW] = xpad.reshape(N, CIN, PW * PW)

    return xflat, w_flat, np.asarray(b, np.float32).reshape(COUT, 1)


_PROGRAM_CACHE = {}


def _build_program(repeat=1, structure="tinner"):
    key = (repeat, structure)
    if key in _PROGRAM_CACHE:
        return _PROGRAM_CACHE[key]

    import concourse.tile as tile
    from concourse import bacc, mybir

    nc = bacc.Bacc("TRN2", target_bir_lowering=False, debug=False)
    F32R = mybir.dt.float32r
    F32 = mybir.dt.float32
    x_d = nc.dram_tensor("x", [NIMG, CIN, XW], F32R, kind="ExternalInput").ap()
    w_d = nc.dram_tensor("w", [128, NOCT * WO], F32R, kind="ExternalInput").ap()
    b_d = nc.dram_tensor("b", [COUT, 1], F32, kind="ExternalInput").ap()
    out_d = nc.dram_tensor("out", [NIMG, COUT, H * W], F32,
                           kind="ExternalOutput").ap()

    from contextlib import nullcontext

    with tile.TileContext(nc) as tc:
        with (
            tc.tile_pool(name="xpool", bufs=2) as xpool,
            tc.tile_pool(name="wpool", bufs=2) as wpool,
            tc.tile_pool(name="accp", bufs=8) as accp,
            tc.tile_pool(name="ps", bufs=(8 if structure == "tinner" else 2),
                         space="PSUM") as ps,
            tc.tile_pool(name="sq", bufs=3) as sqp,
            tc.tile_pool(name="fold", bufs=3) as foldp,
            tc.tile_pool(name="singles", bufs=1) as singles,
        ):
            bt = singles.tile([COUT, 1], F32)
            nc.sync.dma_start(out=bt[:], in_=b_d)

            rep_ctx = (tc.For_i(0, repeat, 1, hint_engines=(mybir.EngineType.PE,))
                       if repeat > 1 else nullcontext())
            with rep_ctx:
                for n in range(NIMG):
                    xt = xpool.tile([128, XW], F32R)
                    nc.sync.dma_start(out=xt[0:64, :], in_=x_d[n])
                    nc.sync.dma_start(out=xt[64:128, 0:XW - 1],
                                      in_=x_d[n, :, 1:XW])

                    accs = [accp.tile([128, 512], F32, name=f"acc{_t}", tag="acc")
                            for _t in range(8)]

                    for o in range(NOCT):
                        wt = wpool.tile([128, WO], F32R)
                        nc.sync.dma_start(out=wt[:],
                                          in_=w_d[:, o * WO:(o + 1) * WO])
                        if structure == "tinner":
                            # weight-stationary: same lhsT across the 8
                            # pixel tiles before moving to the next tap
                            psums = [ps.tile([128, 512], F32, name=f"pst{_i}", tag="pst")
                                     for _i in range(8)]
                            for ky in range(K):
                                for g in range(5):
                                    idx = ky * 5 + g
                                    for t in range(8):
                                        base = (t * 8 + ky) * PW + 2 * g
                                        win = xt[:, base:base + 8 * PW].rearrange(
                                            "p (r c) -> p r c", c=PW)[:, :, 0:64]
                                        nc.tensor.matmul(
                                            psums[t][:],
                                            wt[:, idx * 128:(idx + 1) * 128],
                                            win,
                                            start=(idx == 0), stop=(idx == 44))
                            for t in range(8):
                                if o == 0:
                                    nc.scalar.activation(
                                        accs[t][:], psums[t][:],
                                        mybir.ActivationFunctionType.Square)
                                else:
                                    sq = sqp.tile([128, 512], F32)
                                    nc.scalar.activation(
                                        sq[:], psums[t][:],
                                        mybir.ActivationFunctionType.Square)
                                    nc.vector.tensor_add(accs[t][:], accs[t][:],
                                                         sq[:])
                        else:
                            for t in range(8):
                                y0 = t * 8
                                acc = ps.tile([128, 512], F32)
                                for ky in range(K):
                                    for g in range(5):
                                        idx = ky * 5 + g
                                        base = (y0 + ky) * PW + 2 * g
                                        win = xt[:, base:base + 8 * PW].rearrange(
                                            "p (r c) -> p r c", c=PW)[:, :, 0:64]
                                        nc.tensor.matmul(
                                            acc[:],
                                            wt[:, idx * 128:(idx + 1) * 128],
                                            win,
                                            start=(idx == 0), stop=(idx == 44))
                                if o == 0:
                                    nc.scalar.activation(
                                        accs[t][:], acc[:],
                                        mybir.ActivationFunctionType.Square)
                                else:
                                    sq = sqp.tile([128, 512], F32)
                                    nc.scalar.activation(
                                        sq[:], acc[:],
                                        mybir.ActivationFunctionType.Square)
                                    nc.vector.tensor_add(accs[t][:], accs[t][:],
                                                         sq[:])

                    for t in range(8):
                        tmp = foldp.tile([COUT, 512], F32)
                        nc.vector.tensor_copy(tmp[:], accs[t][64:128, :])
                        f = foldp.tile([COUT, 512], F32)
                        nc.vector.scalar_tensor_tensor(
                            f[:], tmp[:], bt[:], accs[t][0:64, :],
                            op0=mybir.AluOpType.add, op1=mybir.AluOpType.add)
                        nc.sync.dma_start(
                            out=out_d[n, :, t * 512:(t + 1) * 512], in_=f[:])

    nc.compile()
    _PROGRAM_CACHE[key] = nc
    return nc


_RUNNER_CACHE = {}


def _make_runner(nc):
    """Build a reusable jitted 8-core executor for the program `nc`.

    Mirrors bass2jax.run_bass_via_pjrt's multi-core path, but keeps the
    jitted shard_map alive so repeat calls don't re-trace/re-compile.
    """
    import jax
    from jax.experimental.shard_map import shard_map
    from jax.sharding import Mesh, PartitionSpec
    from concourse import bass2jax, mybir

    bass2jax.install_neuronx_cc_hook()

    partition_name = (nc.partition_id_tensor.name
                      if nc.partition_id_tensor else None)
    in_names, out_names, out_avals, out_shapes = [], [], [], []
    for alloc in nc.m.functions[0].allocations:
        if not isinstance(alloc, mybir.MemoryLocationSet):
            continue
        name = alloc.memorylocations[0].name
        if alloc.kind == "ExternalInput":
            if name != partition_name:
                in_names.append(name)
        elif alloc.kind == "ExternalOutput":
            shape = tuple(alloc.tensor_shape)
            dtype = mybir.dt.np(alloc.dtype)
            out_names.append(name)
            out_avals.append(jax.core.ShapedArray(shape, dtype))
            out_shapes.append((shape, dtype))
    n_params = len(in_names)
    n_outs = len(out_names)
    all_in_names = list(in_names) + list(out_names)
    if partition_name is not None:
        all_in_names.append(partition_name)
    donate = tuple(range(n_params, n_params + n_outs))

    def _body(*args):
        operands = list(args)
        if partition_name is not None:
            operands.append(bass2jax.partition_id_tensor())
        outs = bass2jax._bass_exec_p.bind(
            *operands,
            out_avals=tuple(out_avals),
            in_names=tuple(all_in_names),
            out_names=tuple(out_names),
            lowering_input_output_aliases=(),
            sim_require_finite=True,
            sim_require_nnan=True,
            nc=nc,
        )
        return tuple(outs)

    devices = jax.devices()[:N_CORES]
    mesh = Mesh(np.asarray(devices), ("core",))
    in_specs = (PartitionSpec("core"),) * (n_params + n_outs)
    out_specs = (PartitionSpec("core"),) * n_outs
    sharded = jax.jit(
        shard_map(_body, mesh=mesh, in_specs=in_specs, out_specs=out_specs,
                  check_rep=False),
        donate_argnums=donate, keep_unused=True)

    from jax.sharding import NamedSharding
    core_sharding = NamedSharding(mesh, PartitionSpec("core"))
    dev_cache = {}

    def run(in_maps, cache_key=None):
        if cache_key is not None and cache_key in dev_cache:
            concat_in = dev_cache[cache_key]
        else:
            concat_in = [
                jax.device_put(
                    np.concatenate([np.asarray(in_maps[c][name])
                                    for c in range(N_CORES)], axis=0),
                    core_sharding)
                for name in in_names]
            if cache_key is not None:
                dev_cache[cache_key] = concat_in
        concat_zeros = [
            np.zeros((N_CORES * s[0],) + tuple(s[1:]), d)
            for (s, d) in out_shapes]
        out_arrs = sharded(*concat_in, *concat_zeros)
        return [
            {name: np.asarray(out_arrs[i]).reshape(
                (N_CORES,) + out_shapes[i][0])[c]
             for i, name in enumerate(out_names)}
            for c in range(N_CORES)]

    return run


def _run(nc, xflat, w_flat, b_col, cache_key=None):
    runner = _RUNNER_CACHE.get(id(nc))
    if runner is None:
        runner = _make_runner(nc)
        _RUNNER_CACHE[id(nc)] = runner
    in_maps = []
    for c in range(N_CORES):
        in_maps.append({
            "x": np.ascontiguousarray(xflat[c * NIMG:(c + 1) * NIMG]),
            "w": w_flat,
            "b": b_col,
        })
    results = runner(in_maps, cache_key=cache_key)
    out = np.concatenate(
        [results[c]["out"].reshape(NIMG, COUT, H, W)
         for c in range(N_CORES)], axis=0)
    return out


def kernel(x, T_real, T_imag, w_r, w_i, b, _repeat=1, _structure="tinner"):
    x = np.asarray(x, np.float32)
    xflat, w_flat, b_col = _host_prep(
        x, np.asarray(T_real, np.float32), np.asarray(T_imag, np.float32),
        np.asarray(w_r, np.float32), np.asarray(w_i, np.float32), b)
    nc = _build_program(repeat=_repeat, structure=_structure)
    return _run(nc, xflat, w_flat, b_col)

